# revision 12
# baseline (speedup 1.0000x reference)
"""Trainium2 Bass kernel for nn_CotLayer (CoT attention layer, dense_cnn).

Sharding: 8 cores = 4 clips x 2 spatial halves (H rows 0:32 / 32:64).
All convs are 1x1 spatially, so a spatial split needs no halo. Two tiny
pair-wise AllReduces handle the cross-half reductions (GroupNorm stats of
the dynamic-kernel branch, and the split-attention GAP).
"""
import sys
import numpy as np

try:
    import concourse.bass as bass  # noqa: F401
except ImportError:
    sys.path.insert(0, "/opt/trn_rl_repo")

import concourse.bass as bass
import concourse.tile as tile
from concourse import mybir, bacc
from concourse.bass_utils import run_bass_kernel_spmd

# ---- problem constants (hardcoded per spec) ----
C = 128          # channels
NF = 8           # temporal frames per clip
B = 4            # clips
H = W = 64
KS = 3
G = 32           # groupnorm groups = C//4
KC = 96          # KS * C//4 dynamic-kernel channels
EPS = 1e-5
PXC = 2048       # pixels per core (H/2 * W)
PT = 512         # pixel tile
NT = PXC // PT   # 4 tiles
NI = NT * NF     # 32 (tile, frame) iterations
NCORES = 8
PAIRS = [[0, 1], [2, 3], [4, 5], [6, 7]]

F32 = mybir.dt.float32
F32R = mybir.dt.float32r
F16 = mybir.dt.float16
AF = mybir.ActivationFunctionType
ALU = mybir.AluOpType
AXL = mybir.AxisListType

_CACHE = {}


def _build_program():
    nc = bacc.Bacc("TRN2", target_bir_lowering=False, debug=False,
                   num_devices=NCORES)

    def din(name, shape, dt):
        return nc.dram_tensor(name, shape, dt, kind="ExternalInput").ap()

    # big input / output
    x_d = din("x", [C, NF, PXC], F32R)
    out_d = nc.dram_tensor("out", [C, NF, PXC], F32, kind="ExternalOutput").ap()

    # weights (lhsT layouts) and folded biases
    wkey_d = din("wkey", [C, KS, C], F32R)       # key conv per tap, BN folded
    tk_d = din("tk", [C, 1], F32)
    we1a_d = din("we1a", [C, C // 2], F32R)      # e1 (x part), BN folded
    we1b_d = din("we1b", [C, C // 2], F16)       # e1 (k2d part)
    te_d = din("te", [C // 2, 1], F32)
    we2_d = din("we2", [C // 2, KC], F16)        # e2 (raw, b_e2 via stats path)
    wv_d = din("wv", [C, C], F32R)               # value 1x1, BN folded
    tv_d = din("tv", [C, 1], F32)
    bkg_d = din("bkg", [KC, KS, C], F16)         # broadcast masks * gn_g
    gmu_d = din("gmu", [KC, G], F32)             # group-sum * (1/12288)
    c1_d = din("c1", [G, C], F32)                # group -> channel bcast
    c96_d = din("c96", [G, KC], F32)             # group -> 96 bcast
    badj_d = din("badj", [KC, 3], F32)           # b_e2 sum adjust consts
    ecols_d = din("ecols", [C, KS, 3], F32)      # (b_e2, gn_g, gn_b) c-layout
    s2_d = din("s2", [C, 1], F32)                # bn2 scale
    t2_d = din("t2", [C, 1], F32)                # bn2 bias
    wse1_d = din("wse1", [C, C], F32)            # SE matmul 1 (folded)
    b1_d = din("b1", [C, 1], F32)
    wsed_d = din("wsed", [C, C], F32)            # SE delta-logit matmul
    bd_d = din("bd", [C, 1], F32)
    ident_d = din("ident", [C, C], F16)
    epsv_d = din("epsv", [G, 1], F32)

    # collective bounce buffers (internal DRAM)
    cc1_in = nc.dram_tensor("cc1_in", [KC, 16], F32)
    cc1_out = nc.dram_tensor("cc1_out", [KC, 16], F32)
    cc2_in = nc.dram_tensor("cc2_in", [C, 1], F32)
    cc2_out = nc.dram_tensor("cc2_out", [C, 1], F32)

    with tile.TileContext(nc) as tc:
        with tc.tile_pool(name="consts", bufs=1) as cp, \
             tc.tile_pool(name="k2dp", bufs=NI + 1) as k2dp, \
             tc.tile_pool(name="wdp", bufs=NI + 1) as wdp, \
             tc.tile_pool(name="aggp", bufs=NI + 1) as aggp, \
             tc.tile_pool(name="statp", bufs=1) as stp:

            # ---- load constants ----
            def ctile(ap, dt, name):
                t = cp.tile(list(ap.shape), dt, tag=name)
                nc.sync.dma_start(t[:], ap[:])
                return t

            wkey = ctile(wkey_d, F32R, "wkey")
            tk = ctile(tk_d, F32, "tk")
            we1a = ctile(we1a_d, F32R, "we1a")
            we1b = ctile(we1b_d, F16, "we1b")
            te = ctile(te_d, F32, "te")
            we2 = ctile(we2_d, F16, "we2")
            wv = ctile(wv_d, F32R, "wv")
            tv = ctile(tv_d, F32, "tv")
            bkg = ctile(bkg_d, F16, "bkg")
            gmu = ctile(gmu_d, F32, "gmu")
            c1m = ctile(c1_d, F32, "c1m")
            c96m = ctile(c96_d, F32, "c96m")
            badj = ctile(badj_d, F32, "badj")
            ecols = ctile(ecols_d, F32, "ecols")
            s2t = ctile(s2_d, F32, "s2t")
            t2t = ctile(t2_d, F32, "t2t")
            wse1 = ctile(wse1_d, F32, "wse1")
            b1t = ctile(b1_d, F32, "b1t")
            wsed = ctile(wsed_d, F32, "wsed")
            bdt = ctile(bd_d, F32, "bdt")
            ident = ctile(ident_d, F16, "ident")
            epsv = ctile(epsv_d, F32, "epsv")

            # persistent stat buffers
            stats_buf = stp.tile([KC, NT, NF, 6], F32, tag="stats")
            gap_cols = stp.tile([C, 2 * NI], F32, tag="gapc")
            s_all = [stp.tile([KC + 1, NF, C], F16, tag=f"sall{k}",
                              name=f"sall{k}")
                     for k in range(KS)]

            k2d_t = {}
            wd_t = {}
            agg_t = {}

            # ================= PASS A =================
            with tc.tile_pool(name="xwA", bufs=5) as xw, \
                 tc.tile_pool(name="psA", bufs=2, space="PSUM") as psA:
                for t in range(NT):
                    xt = {}

                    def loadx(n, t=t, xt=xt):
                        tl = xw.tile([C, PT], F32R, tag="xA")
                        nc.sync.dma_start(
                            tl[:], x_d[:, n, t * PT:(t + 1) * PT])
                        xt[n] = tl

                    loadx(0)
                    loadx(1)
                    for n in range(NF):
                        if n + 2 < NF:
                            loadx(n + 2)
                        # --- key embed: grouped temporal conv (dense taps) ---
                        ps_k = psA.tile([C, PT], F32, tag="ps_k")
                        taps = [k for k in range(KS) if 0 <= n + k - 1 < NF]
                        for i, k in enumerate(taps):
                            nc.tensor.matmul(
                                ps_k[:], wkey[:, k, :], xt[n + k - 1][:],
                                start=(i == 0), stop=(i == len(taps) - 1))
                        k2 = k2dp.tile([C, PT], F16, tag="k2d")
                        idx = t * NF + n
                        nc.scalar.activation(
                            k2[:], ps_k[:], AF.Relu, bias=tk[:],
                            accum_out=gap_cols[:, idx:idx + 1])
                        k2d_t[(t, n)] = k2
                        # --- e = relu(bn(w_e1 @ [x; k2d])) ---
                        ps_e = psA.tile([C // 2, PT], F32, tag="ps_e")
                        nc.tensor.matmul(ps_e[:], we1a[:], xt[n][:],
                                         start=True, stop=False)
                        nc.tensor.matmul(ps_e[:], we1b[:], k2[:],
                                         start=False, stop=True)
                        et = xw.tile([C // 2, PT], F16, tag="eA")
                        nc.scalar.activation(et[:], ps_e[:], AF.Relu,
                                             bias=te[:])
                        # --- wd = w_e2 @ e (raw; b_e2 handled via stats) ---
                        ps_w = psA.tile([KC, PT], F32, tag="ps_w")
                        nc.tensor.matmul(ps_w[:], we2[:], et[:],
                                         start=True, stop=True)
                        nc.vector.bn_stats(stats_buf[:, t, n, :], ps_w[:])
                        wdt = wdp.tile([KC + 1, PT], F16, tag="wd")
                        nc.scalar.activation(wdt[0:KC, :], ps_w[:], AF.Copy)
                        nc.vector.memset(wdt[KC:KC + 1, :], 1.0)
                        wd_t[(t, n)] = wdt

            # ================= GroupNorm stats + collective =================
            with tc.tile_pool(name="stw", bufs=1) as sw, \
                 tc.tile_pool(name="psS", bufs=1, space="PSUM") as psS:
                mv = sw.tile([KC, NF, 2], F32, tag="mv")
                for n in range(NF):
                    nc.vector.bn_aggr(mv[:, n, :], stats_buf[:, :, n, :])
                # per-channel sums over this core's 2048 px:
                #   S = mean*2048 ; Q = (var + mean^2)*2048
                sums = sw.tile([KC, 16], F32, tag="sums")
                sq = sw.tile([KC, NF], F32, tag="sq")
                nc.vector.tensor_mul(sq[:], mv[:, :, 0], mv[:, :, 0])
                nc.vector.tensor_add(sums[:, 8:16], mv[:, :, 1], sq[:])
                nc.vector.tensor_scalar(
                    out=sums[:, 8:16], in0=sums[:, 8:16],
                    scalar1=float(PXC), scalar2=None, op0=ALU.mult)
                nc.vector.tensor_scalar(
                    out=sums[:, 0:8], in0=mv[:, :, 0],
                    scalar1=float(PXC), scalar2=None, op0=ALU.mult)
                # adjust for the (unapplied) conv bias b_e2:
                #   S' = S + 2048*b ; Q' = Q + 2b*S + 2048*b^2
                tmp = sw.tile([KC, NF], F32, tag="tmpb")
                nc.vector.tensor_scalar(
                    out=tmp[:], in0=sums[:, 0:8], scalar1=badj[:, 1:2],
                    scalar2=None, op0=ALU.mult)
                nc.vector.tensor_add(sums[:, 8:16], sums[:, 8:16], tmp[:])
                nc.vector.tensor_scalar(
                    out=sums[:, 8:16], in0=sums[:, 8:16],
                    scalar1=badj[:, 2:3], scalar2=None, op0=ALU.add)
                nc.vector.tensor_scalar(
                    out=sums[:, 0:8], in0=sums[:, 0:8],
                    scalar1=badj[:, 0:1], scalar2=None, op0=ALU.add)
                # pair AllReduce -> full-frame sums
                nc.sync.dma_start(cc1_in.ap()[:], sums[:])
                nc.gpsimd.collective_compute(
                    "AllReduce", ALU.add, replica_groups=PAIRS,
                    ins=[cc1_in.ap()], outs=[cc1_out.ap()])
                sums_all = sw.tile([KC, 16], F32, tag="sums_all")
                nc.sync.dma_start(sums_all[:], cc1_out.ap()[:])
                # group stats: mu/E[x^2] (both /12288 via gmu)
                ps_g = psS.tile([G, 16], F32, tag="ps_g")
                nc.tensor.matmul(ps_g[:], gmu[:], sums_all[:],
                                 start=True, stop=True)
                mv32 = sw.tile([G, 16], F32, tag="mv32")
                nc.scalar.activation(mv32[:, 0:8], ps_g[:, 0:8], AF.Copy)
                var = sw.tile([G, NF], F32, tag="var")
                nc.vector.tensor_mul(var[:], mv32[:, 0:8], mv32[:, 0:8])
                nc.vector.tensor_sub(var[:], ps_g[:, 8:16], var[:])
                nc.scalar.activation(var[:], var[:], AF.Sqrt, bias=epsv[:])
                nc.vector.reciprocal(mv32[:, 8:16], var[:])
                # broadcast stats: groups -> 96 rows and -> 128 channels
                ps96 = psS.tile([KC, NF], F32, tag="ps96")
                nc.tensor.matmul(ps96[:], c96m[:], mv32[:, 8:16],
                                 start=True, stop=True)
                rs96 = sw.tile([KC, NF], F32, tag="rs96")
                nc.scalar.activation(rs96[:], ps96[:], AF.Copy)
                psc1 = psS.tile([C, 16], F32, tag="psc1")
                nc.tensor.matmul(psc1[:], c1m[:], mv32[:],
                                 start=True, stop=True)
                mbrb = sw.tile([C, 16], F32, tag="mbrb")
                nc.scalar.activation(mbrb[:], psc1[:], AF.Copy)
                # t-bias in channel layout: t[c,k,n] =
                #   (b_e2[gk(c)] - mu[c]) * rs[c] * gn_g[gk(c)] + gn_b[gk(c)]
                tb = sw.tile([C, KS, NF], F16, tag="tb")
                tba = sw.tile([C, NF], F32, tag="tba")
                tbb = sw.tile([C, NF], F32, tag="tbb")
                for k in range(KS):
                    nc.vector.tensor_scalar(
                        out=tba[:], in0=mbrb[:, 0:8], scalar1=-1.0,
                        scalar2=ecols[:, k, 0:1], op0=ALU.mult, op1=ALU.add)
                    nc.vector.tensor_mul(tbb[:], tba[:], mbrb[:, 8:16])
                    nc.vector.tensor_scalar(
                        out=tbb[:], in0=tbb[:], scalar1=ecols[:, k, 1:2],
                        scalar2=None, op0=ALU.mult)
                    nc.vector.tensor_scalar(
                        out=tb[:, k, :], in0=tbb[:], scalar1=ecols[:, k, 2:3],
                        scalar2=None, op0=ALU.add)
                # build S_all_k: rows 0:96 scaled masks, row 96 = t-bias row
                for k in range(KS):
                    for n in range(NF):
                        nc.vector.tensor_scalar(
                            out=s_all[k][0:KC, n, :], in0=bkg[:, k, :],
                            scalar1=rs96[:, n:n + 1], scalar2=None,
                            op0=ALU.mult)
                        nc.sync.dma_start(s_all[k][KC:KC + 1, n, :],
                                          tb[:, k, n:n + 1])

            # ================= PASS B =================
            with tc.tile_pool(name="xwB", bufs=5) as xwB, \
                 tc.tile_pool(name="vw", bufs=6) as vw, \
                 tc.tile_pool(name="mw", bufs=4) as mw, \
                 tc.tile_pool(name="psB", bufs=2, space="PSUM") as psB:
                for t in range(NT):
                    xt = {}
                    vt = {}

                    def loadx(n, t=t, xt=xt):
                        tl = xwB.tile([C, PT], F32R, tag="xB")
                        nc.sync.dma_start(
                            tl[:], x_d[:, n, t * PT:(t + 1) * PT])
                        xt[n] = tl

                    def makev(n, t=t, xt=xt, vt=vt):
                        ps_v = psB.tile([C, PT], F32, tag="ps_v")
                        nc.tensor.matmul(ps_v[:], wv[:], xt[n][:],
                                         start=True, stop=True)
                        tl = vw.tile([C, PT], F32, tag="v")
                        nc.scalar.activation(tl[:], ps_v[:], AF.Identity,
                                             bias=tv[:])
                        vt[n] = tl

                    loadx(0)
                    loadx(1)
                    makev(0)
                    makev(1)
                    for n in range(NF):
                        if n + 2 < NF:
                            loadx(n + 2)
                            makev(n + 2)
                        taps = [k for k in range(KS) if 0 <= n + k - 1 < NF]
                        terms = []
                        for k in taps:
                            ps_w = psB.tile([C, PT], F32, tag=f"ps_w{k}")
                            nc.tensor.matmul(
                                ps_w[:], s_all[k][:, n, :],
                                wd_t[(t, n)][:], start=True, stop=True)
                            m = mw.tile([C, PT], F32, tag=f"m{k}")
                            nc.vector.tensor_mul(m[:], ps_w[:],
                                                 vt[n + k - 1][:])
                            terms.append(m)
                        acc = terms[0]
                        for m in terms[1:]:
                            a2 = mw.tile([C, PT], F32, tag="accB")
                            nc.gpsimd.tensor_add(a2[:], acc[:], m[:])
                            acc = a2
                        agg = aggp.tile([C, PT], F16, tag="agg")
                        idx = t * NF + n
                        nc.scalar.activation(
                            agg[:], acc[:], AF.Silu, bias=t2t[:],
                            scale=s2t[:],
                            accum_out=gap_cols[:, NI + idx:NI + idx + 1])
                        agg_t[(t, n)] = agg

            # ================= GAP + SE attention =================
            with tc.tile_pool(name="sew", bufs=1) as se, \
                 tc.tile_pool(name="psE", bufs=1, space="PSUM") as psE:
                gap = se.tile([C, 1], F32, tag="gap")
                nc.vector.tensor_reduce(gap[:], gap_cols[:], AXL.XYZW,
                                        ALU.add)
                nc.sync.dma_start(cc2_in.ap()[:], gap[:])
                nc.gpsimd.collective_compute(
                    "AllReduce", ALU.add, replica_groups=PAIRS,
                    ins=[cc2_in.ap()], outs=[cc2_out.ap()])
                gap_all = se.tile([C, 1], F32, tag="gap_all")
                nc.sync.dma_start(gap_all[:], cc2_out.ap()[:])
                ps_a = psE.tile([C, 1], F32, tag="ps_a")
                nc.tensor.matmul(ps_a[:], wse1[:], gap_all[:],
                                 start=True, stop=True)
                at = se.tile([C, 1], F32, tag="at")
                nc.scalar.activation(at[:], ps_a[:], AF.Relu, bias=b1t[:])
                ps_d = psE.tile([C, 1], F32, tag="ps_d")
                nc.tensor.matmul(ps_d[:], wsed[:], at[:],
                                 start=True, stop=True)
                sa = se.tile([C, 1], F32, tag="sa")
                nc.scalar.activation(sa[:], ps_d[:], AF.Sigmoid, bias=bdt[:])
                sk = se.tile([C, 1], F32, tag="sk")
                nc.vector.tensor_scalar(out=sk[:], in0=sa[:], scalar1=-1.0,
                                        scalar2=1.0, op0=ALU.mult,
                                        op1=ALU.add)
                diag_a = se.tile([C, C], F16, tag="diag_a")
                diag_k = se.tile([C, C], F16, tag="diag_k")
                nc.vector.tensor_scalar(out=diag_a[:], in0=ident[:],
                                        scalar1=sa[:], scalar2=None,
                                        op0=ALU.mult)
                nc.vector.tensor_scalar(out=diag_k[:], in0=ident[:],
                                        scalar1=sk[:], scalar2=None,
                                        op0=ALU.mult)

                # ================= PASS C =================
                with tc.tile_pool(name="ow", bufs=3) as ow, \
                     tc.tile_pool(name="psC", bufs=3, space="PSUM") as psC:
                    for t in range(NT):
                        for n in range(NF):
                            ps_o = psC.tile([C, PT], F32, tag="ps_o")
                            nc.tensor.matmul(ps_o[:], diag_a[:],
                                             agg_t[(t, n)][:],
                                             start=True, stop=False)
                            nc.tensor.matmul(ps_o[:], diag_k[:],
                                             k2d_t[(t, n)][:],
                                             start=False, stop=True)
                            ot = ow.tile([C, PT], F32, tag="ot")
                            nc.scalar.activation(ot[:], ps_o[:], AF.Copy)
                            nc.sync.dma_start(
                                out_d[:, n, t * PT:(t + 1) * PT], ot[:])

    nc.compile()
    return nc


def _host_constants(inp):
    f = np.float32
    d = {}
    s_k = (inp["bnk_g"] / np.sqrt(inp["bnk_v"] + EPS)).astype(f)
    t_k = (inp["bnk_b"] - inp["bnk_m"] * s_k).astype(f)
    w_key = inp["w_key"].reshape(C, C // 4, KS)          # (o, i_local, tap)
    wk = np.zeros((KS, C, C), f)
    for o in range(C):
        g = o // 32
        wk[:, 32 * g:32 * (g + 1), o] = (w_key[o].T * s_k[o])
    d["wkey"] = np.ascontiguousarray(wk.transpose(1, 0, 2))  # (i, tap, o)
    d["tk"] = t_k.reshape(C, 1)

    s_e = (inp["bne_g"] / np.sqrt(inp["bne_v"] + EPS)).astype(f)
    t_e = (inp["bne_b"] - inp["bne_m"] * s_e).astype(f)
    we1 = inp["w_e1"] * s_e[:, None]                      # (64, 256)
    d["we1a"] = np.ascontiguousarray(we1[:, :C].T).astype(f)
    d["we1b"] = np.ascontiguousarray(we1[:, C:].T).astype(np.float16)
    d["te"] = t_e.reshape(C // 2, 1)
    d["we2"] = np.ascontiguousarray(inp["w_e2"].T).astype(np.float16)

    s_1 = (inp["bn1_g"] / np.sqrt(inp["bn1_v"] + EPS)).astype(f)
    t_1 = (inp["bn1_b"] - inp["bn1_m"] * s_1).astype(f)
    d["wv"] = np.ascontiguousarray((inp["w_1x1"] * s_1[:, None]).T).astype(f)
    d["tv"] = t_1.reshape(C, 1)

    gn_g, gn_b, b_e2 = inp["gn_g"], inp["gn_b"], inp["b_e2"]
    rows = np.arange(KC)
    cols = np.arange(C)
    bkg = np.zeros((KS, KC, C), f)
    for k in range(KS):
        bkg[k] = (rows[:, None] == (3 * (cols[None, :] // 4) + k)) * \
            gn_g[rows][:, None]
    d["bkg"] = np.ascontiguousarray(
        bkg.transpose(1, 0, 2)).astype(np.float16)   # (r, k, c)
    d["gmu"] = ((rows[:, None] // 3 == np.arange(G)[None, :]) /
                np.float32(KS * H * W)).astype(f)
    d["c1"] = (np.arange(G)[:, None] == (cols[None, :] // 4)).astype(f)
    d["c96"] = (np.arange(G)[:, None] == (rows[None, :] // 3)).astype(f)
    d["badj"] = np.stack([PXC * b_e2, 2.0 * b_e2, PXC * b_e2 * b_e2],
                         axis=1).astype(f)
    ge = 3 * (cols // 4)
    ecols = np.zeros((C, KS, 3), f)
    for k in range(KS):
        ecols[:, k, 0] = b_e2[ge + k]
        ecols[:, k, 1] = gn_g[ge + k]
        ecols[:, k, 2] = gn_b[ge + k]
    d["ecols"] = ecols

    s_2 = (inp["bn2_g"] / np.sqrt(inp["bn2_v"] + EPS)).astype(f)
    d["s2"] = s_2.reshape(C, 1)
    d["t2"] = (inp["bn2_b"] - inp["bn2_m"] * s_2).astype(f).reshape(C, 1)

    s_se = (inp["bnse_g"] / np.sqrt(inp["bnse_v"] + EPS)).astype(f)
    # gap in reference = mean over (N,H,W) of sum over 2 branches; our gap
    # accumulates the raw sum of (agg+k2d) over the clip = 32768 terms
    wse1 = inp["w_se1"] * (s_se[:, None] / np.float32(NF * H * W))
    d["wse1"] = np.ascontiguousarray(wse1.T).astype(f)
    d["b1"] = (s_se * inp["b_se1"] +
               (inp["bnse_b"] - inp["bnse_m"] * s_se)).astype(f).reshape(C, 1)
    w2 = inp["w_se2"]
    d["wsed"] = np.ascontiguousarray((w2[0::2, :] - w2[1::2, :]).T).astype(f)
    d["bd"] = (inp["b_se2"][0::2] - inp["b_se2"][1::2]).astype(f).reshape(C, 1)
    d["ident"] = np.eye(C, dtype=np.float16)
    d["epsv"] = np.full((G, 1), EPS, f)
    return d


def kernel(**inputs):
    if "nc" not in _CACHE:
        _CACHE["nc"] = _build_program()
    nc = _CACHE["nc"]

    consts = _host_constants(inputs)
    x = np.ascontiguousarray(inputs["x"].astype(np.float32))
    x5 = x.reshape(B, NF, C, H, W)

    in_maps = []
    for core in range(NCORES):
        clip, q = core // 2, core % 2
        xs = x5[clip][:, :, q * 32:(q + 1) * 32, :]          # (NF, C, 32, W)
        xs = np.ascontiguousarray(
            xs.transpose(1, 0, 2, 3).reshape(C, NF, PXC))
        m = dict(consts)
        m["x"] = xs
        in_maps.append(m)

    res = run_bass_kernel_spmd(nc, in_maps, list(range(NCORES)))

    out = np.empty((B, NF, C, H, W), np.float32)
    for core in range(NCORES):
        clip, q = core // 2, core % 2
        o = res.results[core]["out"].reshape(C, NF, 32, W)
        out[clip][:, :, q * 32:(q + 1) * 32, :] = o.transpose(1, 0, 2, 3)
    return out.reshape(B * NF, C, H, W)


if __name__ == "__main__":
    sys.path.insert(0, "/root/problem")
    import reference
    inp = {k: np.asarray(v) for k, v in reference.setup_inputs().items()}
    got = kernel(**inp)
    exp = np.asarray(reference.reference(**inp))
    err = np.abs(got - exp).max() / np.abs(exp).max()
    print("abs-max relative error:", err)


# revision 17
# speedup vs baseline: 1.4296x; 1.4296x over previous
"""Trainium2 Bass kernel for nn_CotLayer (CoT attention layer, dense_cnn).

Sharding: 8 cores = 4 clips x 2 spatial halves (H rows 0:32 / 32:64).
All convs are 1x1 spatially, so a spatial split needs no halo. Two tiny
pair-wise AllReduces handle the cross-half reductions (GroupNorm stats of
the dynamic-kernel branch, and the split-attention GAP).
"""
import sys
import numpy as np

try:
    import concourse.bass as bass  # noqa: F401
except ImportError:
    sys.path.insert(0, "/opt/trn_rl_repo")

import concourse.bass as bass
import concourse.tile as tile
from concourse import mybir, bacc
from concourse.bass_utils import run_bass_kernel_spmd

# ---- problem constants (hardcoded per spec) ----
C = 128          # channels
NF = 8           # temporal frames per clip
B = 4            # clips
H = W = 64
KS = 3
G = 32           # groupnorm groups = C//4
KC = 96          # KS * C//4 dynamic-kernel channels
EPS = 1e-5
PXC = 2048       # pixels per core (H/2 * W)
PT = 512         # pixel tile
NT = PXC // PT   # 4 tiles
NI = NT * NF     # 32 (tile, frame) iterations
NCORES = 8
PAIRS = [[0, 1], [2, 3], [4, 5], [6, 7]]

F32 = mybir.dt.float32
F32R = mybir.dt.float32r
F16 = mybir.dt.float16
AF = mybir.ActivationFunctionType
ALU = mybir.AluOpType
AXL = mybir.AxisListType

_CACHE = {}


def _build_program(single=False, use_cc=None):
    if use_cc is None:
        use_cc = not single
    nc = bacc.Bacc("TRN2", target_bir_lowering=False, debug=False,
                   num_devices=1 if single else NCORES)

    def din(name, shape, dt):
        return nc.dram_tensor(name, shape, dt, kind="ExternalInput").ap()

    # big input / output
    x_d = din("x", [C, NF, PXC], F32R)
    out_d = nc.dram_tensor("out", [C, NF, PXC], F32, kind="ExternalOutput").ap()

    # weights (lhsT layouts) and folded biases
    wkey_d = din("wkey", [C, KS, C], F32R)       # key conv per tap, BN folded
    tk_d = din("tk", [C, 1], F32)
    we1a_d = din("we1a", [C, C // 2], F32R)      # e1 (x part), BN folded
    we1b_d = din("we1b", [C, C // 2], F16)       # e1 (k2d part)
    te_d = din("te", [C // 2, 1], F32)
    we2_d = din("we2", [C // 2, KC], F16)        # e2 (raw, b_e2 via stats path)
    wv_d = din("wv", [C, C], F32R)               # value 1x1, BN folded
    tv_d = din("tv", [C, 1], F32)
    bkg_d = din("bkg", [KC, KS, C], F16)         # broadcast masks * gn_g
    gmu_d = din("gmu", [KC, G], F32)             # group-sum * (1/12288)
    c1_d = din("c1", [G, C], F32)                # group -> channel bcast
    c96_d = din("c96", [G, KC], F32)             # group -> 96 bcast
    badj_d = din("badj", [KC, 3], F32)           # b_e2 sum adjust consts
    ecols_d = din("ecols", [C, KS, 3], F32)      # (b_e2, gn_g, gn_b) c-layout
    s2_d = din("s2", [C, 1], F32)                # bn2 scale
    t2_d = din("t2", [C, 1], F32)                # bn2 bias
    wse1_d = din("wse1", [C, C], F32)            # SE matmul 1 (folded)
    b1_d = din("b1", [C, 1], F32)
    wsed_d = din("wsed", [C, C], F32)            # SE delta-logit matmul
    bd_d = din("bd", [C, 1], F32)
    ident_d = din("ident", [C, C], F16)
    epsv_d = din("epsv", [G, 1], F32)

    # collective bounce buffers (internal DRAM)
    cc1_in = nc.dram_tensor("cc1_in", [KC, 16], F32)
    cc1_out = nc.dram_tensor("cc1_out", [KC, 16], F32)
    cc2_in = nc.dram_tensor("cc2_in", [C, 1], F32)
    cc2_out = nc.dram_tensor("cc2_out", [C, 1], F32)

    with tile.TileContext(nc) as tc:
        with tc.tile_pool(name="consts", bufs=1) as cp, \
             tc.tile_pool(name="k2dp", bufs=NI + 1) as k2dp, \
             tc.tile_pool(name="wdp", bufs=NI + 1) as wdp, \
             tc.tile_pool(name="aggp", bufs=NI + 1) as aggp, \
             tc.tile_pool(name="statp", bufs=1) as stp:

            # ---- load constants ----
            def ctile(ap, dt, name):
                t = cp.tile(list(ap.shape), dt, tag=name)
                nc.sync.dma_start(t[:], ap[:])
                return t

            wkey = ctile(wkey_d, F32R, "wkey")
            tk = ctile(tk_d, F32, "tk")
            we1a = ctile(we1a_d, F32R, "we1a")
            we1b = ctile(we1b_d, F16, "we1b")
            te = ctile(te_d, F32, "te")
            we2 = ctile(we2_d, F16, "we2")
            wv = ctile(wv_d, F32R, "wv")
            tv = ctile(tv_d, F32, "tv")
            bkg = ctile(bkg_d, F16, "bkg")
            gmu = ctile(gmu_d, F32, "gmu")
            c1m = ctile(c1_d, F32, "c1m")
            c96m = ctile(c96_d, F32, "c96m")
            badj = ctile(badj_d, F32, "badj")
            ecols = ctile(ecols_d, F32, "ecols")
            s2t = ctile(s2_d, F32, "s2t")
            t2t = ctile(t2_d, F32, "t2t")
            wse1 = ctile(wse1_d, F32, "wse1")
            b1t = ctile(b1_d, F32, "b1t")
            wsed = ctile(wsed_d, F32, "wsed")
            bdt = ctile(bd_d, F32, "bdt")
            ident = ctile(ident_d, F16, "ident")
            epsv = ctile(epsv_d, F32, "epsv")

            # persistent stat buffers
            stats_buf = stp.tile([KC, NT, NF, 6], F32, tag="stats")
            gap_cols = stp.tile([C, 2 * NI], F32, tag="gapc")
            s_all = [stp.tile([KC + 1, NF, C], F16, tag=f"sall{k}",
                              name=f"sall{k}")
                     for k in range(KS)]

            k2d_t = {}
            wd_t = {}
            agg_t = {}

            # ================= PASS A =================
            with tc.tile_pool(name="xwA", bufs=5) as xw, \
                 tc.tile_pool(name="psA", bufs=2, space="PSUM") as psA:
                for t in range(NT):
                    xt = {}

                    def loadx(n, t=t, xt=xt):
                        tl = xw.tile([C, PT], F32R, tag="xA")
                        nc.sync.dma_start(
                            tl[:], x_d[:, n, t * PT:(t + 1) * PT])
                        xt[n] = tl

                    loadx(0)
                    loadx(1)
                    for n in range(NF):
                        if n + 2 < NF:
                            loadx(n + 2)
                        # --- key embed: grouped temporal conv (dense taps) ---
                        ps_k = psA.tile([C, PT], F32, tag="ps_k")
                        taps = [k for k in range(KS) if 0 <= n + k - 1 < NF]
                        for i, k in enumerate(taps):
                            nc.tensor.matmul(
                                ps_k[:], wkey[:, k, :], xt[n + k - 1][:],
                                start=(i == 0), stop=(i == len(taps) - 1))
                        k2 = k2dp.tile([C, PT], F16, tag="k2d")
                        idx = t * NF + n
                        nc.scalar.activation(
                            k2[:], ps_k[:], AF.Relu, bias=tk[:],
                            accum_out=gap_cols[:, idx:idx + 1])
                        k2d_t[(t, n)] = k2
                        # --- e = relu(bn(w_e1 @ [x; k2d])) ---
                        ps_e = psA.tile([C // 2, PT], F32, tag="ps_e")
                        nc.tensor.matmul(ps_e[:], we1a[:], xt[n][:],
                                         start=True, stop=False)
                        nc.tensor.matmul(ps_e[:], we1b[:], k2[:],
                                         start=False, stop=True)
                        et = xw.tile([C // 2, PT], F16, tag="eA")
                        nc.scalar.activation(et[:], ps_e[:], AF.Relu,
                                             bias=te[:])
                        # --- wd = w_e2 @ e (raw; b_e2 handled via stats) ---
                        ps_w = psA.tile([KC, PT], F32, tag="ps_w")
                        nc.tensor.matmul(ps_w[:], we2[:], et[:],
                                         start=True, stop=True)
                        nc.vector.bn_stats(stats_buf[:, t, n, :], ps_w[:])
                        wdt = wdp.tile([KC + 1, PT], F16, tag="wd")
                        nc.scalar.activation(wdt[0:KC, :], ps_w[:], AF.Copy)
                        nc.vector.memset(wdt[KC:KC + 1, :], 1.0)
                        wd_t[(t, n)] = wdt

            # ================= GroupNorm stats + collective =================
            with tc.tile_pool(name="stw", bufs=1) as sw, \
                 tc.tile_pool(name="psS", bufs=1, space="PSUM") as psS:
                mv = sw.tile([KC, NF, 2], F32, tag="mv")
                for n in range(NF):
                    nc.vector.bn_aggr(mv[:, n, :], stats_buf[:, :, n, :])
                # per-channel sums over this core's 2048 px:
                #   S = mean*2048 ; Q = (var + mean^2)*2048
                sums = sw.tile([KC, 16], F32, tag="sums")
                sq = sw.tile([KC, NF], F32, tag="sq")
                nc.vector.tensor_mul(sq[:], mv[:, :, 0], mv[:, :, 0])
                nc.vector.tensor_add(sums[:, 8:16], mv[:, :, 1], sq[:])
                nc.vector.tensor_scalar(
                    out=sums[:, 8:16], in0=sums[:, 8:16],
                    scalar1=float(PXC), scalar2=None, op0=ALU.mult)
                nc.vector.tensor_scalar(
                    out=sums[:, 0:8], in0=mv[:, :, 0],
                    scalar1=float(PXC), scalar2=None, op0=ALU.mult)
                # adjust for the (unapplied) conv bias b_e2:
                #   S' = S + 2048*b ; Q' = Q + 2b*S + 2048*b^2
                tmp = sw.tile([KC, NF], F32, tag="tmpb")
                nc.vector.tensor_scalar(
                    out=tmp[:], in0=sums[:, 0:8], scalar1=badj[:, 1:2],
                    scalar2=None, op0=ALU.mult)
                nc.vector.tensor_add(sums[:, 8:16], sums[:, 8:16], tmp[:])
                nc.vector.tensor_scalar(
                    out=sums[:, 8:16], in0=sums[:, 8:16],
                    scalar1=badj[:, 2:3], scalar2=None, op0=ALU.add)
                nc.vector.tensor_scalar(
                    out=sums[:, 0:8], in0=sums[:, 0:8],
                    scalar1=badj[:, 0:1], scalar2=None, op0=ALU.add)
                # pair AllReduce -> full-frame sums
                nc.sync.dma_start(cc1_in.ap()[:], sums[:])
                if not use_cc:
                    nc.gpsimd.dma_start(cc1_out.ap()[:], cc1_in.ap()[:])
                else:
                    nc.gpsimd.collective_compute(
                        "AllReduce", ALU.add, replica_groups=PAIRS,
                        ins=[cc1_in.ap()], outs=[cc1_out.ap()])
                sums_all = sw.tile([KC, 16], F32, tag="sums_all")
                nc.sync.dma_start(sums_all[:], cc1_out.ap()[:])
                # group stats: mu/E[x^2] (both /12288 via gmu)
                ps_g = psS.tile([G, 16], F32, tag="ps_g")
                nc.tensor.matmul(ps_g[:], gmu[:], sums_all[:],
                                 start=True, stop=True)
                mv32 = sw.tile([G, 16], F32, tag="mv32")
                nc.scalar.activation(mv32[:, 0:8], ps_g[:, 0:8], AF.Copy)
                var = sw.tile([G, NF], F32, tag="var")
                nc.vector.tensor_mul(var[:], mv32[:, 0:8], mv32[:, 0:8])
                nc.vector.tensor_sub(var[:], ps_g[:, 8:16], var[:])
                nc.scalar.activation(var[:], var[:], AF.Sqrt, bias=epsv[:])
                nc.vector.reciprocal(mv32[:, 8:16], var[:])
                # broadcast stats: groups -> 96 rows and -> 128 channels
                ps96 = psS.tile([KC, NF], F32, tag="ps96")
                nc.tensor.matmul(ps96[:], c96m[:], mv32[:, 8:16],
                                 start=True, stop=True)
                rs96 = sw.tile([KC, NF], F32, tag="rs96")
                nc.scalar.activation(rs96[:], ps96[:], AF.Copy)
                psc1 = psS.tile([C, 16], F32, tag="psc1")
                nc.tensor.matmul(psc1[:], c1m[:], mv32[:],
                                 start=True, stop=True)
                mbrb = sw.tile([C, 16], F32, tag="mbrb")
                nc.scalar.activation(mbrb[:], psc1[:], AF.Copy)
                # t-bias in channel layout: t[c,k,n] =
                #   (b_e2[gk(c)] - mu[c]) * rs[c] * gn_g[gk(c)] + gn_b[gk(c)]
                tb = sw.tile([C, KS, NF], F16, tag="tb")
                tba = sw.tile([C, NF], F32, tag="tba")
                tbb = sw.tile([C, NF], F32, tag="tbb")
                for k in range(KS):
                    nc.vector.tensor_scalar(
                        out=tba[:], in0=mbrb[:, 0:8], scalar1=-1.0,
                        scalar2=ecols[:, k, 0:1], op0=ALU.mult, op1=ALU.add)
                    nc.vector.tensor_mul(tbb[:], tba[:], mbrb[:, 8:16])
                    nc.vector.tensor_scalar(
                        out=tbb[:], in0=tbb[:], scalar1=ecols[:, k, 1:2],
                        scalar2=None, op0=ALU.mult)
                    nc.vector.tensor_scalar(
                        out=tb[:, k, :], in0=tbb[:], scalar1=ecols[:, k, 2:3],
                        scalar2=None, op0=ALU.add)
                # build S_all_k: rows 0:96 scaled masks, row 96 = t-bias row
                for k in range(KS):
                    for n in range(NF):
                        nc.vector.tensor_scalar(
                            out=s_all[k][0:KC, n, :], in0=bkg[:, k, :],
                            scalar1=rs96[:, n:n + 1], scalar2=None,
                            op0=ALU.mult)
                        nc.sync.dma_start(s_all[k][KC:KC + 1, n, :],
                                          tb[:, k, n:n + 1])

            # ================= PASS B =================
            with tc.tile_pool(name="xwB", bufs=5) as xwB, \
                 tc.tile_pool(name="vw", bufs=6) as vw, \
                 tc.tile_pool(name="mw", bufs=4) as mw, \
                 tc.tile_pool(name="psB", bufs=2, space="PSUM") as psB:
                for t in range(NT):
                    xt = {}
                    vt = {}

                    def loadx(n, t=t, xt=xt):
                        tl = xwB.tile([C, PT], F32R, tag="xB")
                        nc.sync.dma_start(
                            tl[:], x_d[:, n, t * PT:(t + 1) * PT])
                        xt[n] = tl

                    def makev(n, t=t, xt=xt, vt=vt):
                        ps_v = psB.tile([C, PT], F32, tag="ps_v")
                        nc.tensor.matmul(ps_v[:], wv[:], xt[n][:],
                                         start=True, stop=True)
                        tl = vw.tile([C, PT], F32, tag="v")
                        nc.scalar.activation(tl[:], ps_v[:], AF.Identity,
                                             bias=tv[:])
                        vt[n] = tl

                    loadx(0)
                    loadx(1)
                    makev(0)
                    makev(1)
                    for n in range(NF):
                        if n + 2 < NF:
                            loadx(n + 2)
                            makev(n + 2)
                        taps = [k for k in range(KS) if 0 <= n + k - 1 < NF]
                        terms = []
                        for k in taps:
                            ps_w = psB.tile([C, PT], F32, tag=f"ps_w{k}")
                            nc.tensor.matmul(
                                ps_w[:], s_all[k][:, n, :],
                                wd_t[(t, n)][:], start=True, stop=True)
                            m = mw.tile([C, PT], F32, tag=f"m{k}")
                            nc.vector.tensor_mul(m[:], ps_w[:],
                                                 vt[n + k - 1][:])
                            terms.append(m)
                        acc = terms[0]
                        for m in terms[1:]:
                            a2 = mw.tile([C, PT], F32, tag="accB")
                            nc.gpsimd.tensor_add(a2[:], acc[:], m[:])
                            acc = a2
                        agg = aggp.tile([C, PT], F16, tag="agg")
                        idx = t * NF + n
                        nc.scalar.activation(
                            agg[:], acc[:], AF.Silu, bias=t2t[:],
                            scale=s2t[:],
                            accum_out=gap_cols[:, NI + idx:NI + idx + 1])
                        agg_t[(t, n)] = agg

            # ================= GAP + SE attention =================
            with tc.tile_pool(name="sew", bufs=1) as se, \
                 tc.tile_pool(name="psE", bufs=1, space="PSUM") as psE:
                gap = se.tile([C, 1], F32, tag="gap")
                nc.vector.tensor_reduce(gap[:], gap_cols[:], AXL.XYZW,
                                        ALU.add)
                nc.sync.dma_start(cc2_in.ap()[:], gap[:])
                if not use_cc:
                    nc.gpsimd.dma_start(cc2_out.ap()[:], cc2_in.ap()[:])
                else:
                    nc.gpsimd.collective_compute(
                        "AllReduce", ALU.add, replica_groups=PAIRS,
                        ins=[cc2_in.ap()], outs=[cc2_out.ap()])
                gap_all = se.tile([C, 1], F32, tag="gap_all")
                nc.sync.dma_start(gap_all[:], cc2_out.ap()[:])
                ps_a = psE.tile([C, 1], F32, tag="ps_a")
                nc.tensor.matmul(ps_a[:], wse1[:], gap_all[:],
                                 start=True, stop=True)
                at = se.tile([C, 1], F32, tag="at")
                nc.scalar.activation(at[:], ps_a[:], AF.Relu, bias=b1t[:])
                ps_d = psE.tile([C, 1], F32, tag="ps_d")
                nc.tensor.matmul(ps_d[:], wsed[:], at[:],
                                 start=True, stop=True)
                sa = se.tile([C, 1], F32, tag="sa")
                nc.scalar.activation(sa[:], ps_d[:], AF.Sigmoid, bias=bdt[:])
                sk = se.tile([C, 1], F32, tag="sk")
                nc.vector.tensor_scalar(out=sk[:], in0=sa[:], scalar1=-1.0,
                                        scalar2=1.0, op0=ALU.mult,
                                        op1=ALU.add)
                diag_a = se.tile([C, C], F16, tag="diag_a")
                diag_k = se.tile([C, C], F16, tag="diag_k")
                nc.vector.tensor_scalar(out=diag_a[:], in0=ident[:],
                                        scalar1=sa[:], scalar2=None,
                                        op0=ALU.mult)
                nc.vector.tensor_scalar(out=diag_k[:], in0=ident[:],
                                        scalar1=sk[:], scalar2=None,
                                        op0=ALU.mult)

                # ================= PASS C =================
                with tc.tile_pool(name="ow", bufs=3) as ow, \
                     tc.tile_pool(name="psC", bufs=3, space="PSUM") as psC:
                    for t in range(NT):
                        for n in range(NF):
                            ps_o = psC.tile([C, PT], F32, tag="ps_o")
                            nc.tensor.matmul(ps_o[:], diag_a[:],
                                             agg_t[(t, n)][:],
                                             start=True, stop=False)
                            nc.tensor.matmul(ps_o[:], diag_k[:],
                                             k2d_t[(t, n)][:],
                                             start=False, stop=True)
                            ot = ow.tile([C, PT], F32, tag="ot")
                            nc.scalar.activation(ot[:], ps_o[:], AF.Copy)
                            nc.sync.dma_start(
                                out_d[:, n, t * PT:(t + 1) * PT], ot[:])

    nc.compile()
    return nc


def _host_constants(inp):
    f = np.float32
    d = {}
    s_k = (inp["bnk_g"] / np.sqrt(inp["bnk_v"] + EPS)).astype(f)
    t_k = (inp["bnk_b"] - inp["bnk_m"] * s_k).astype(f)
    w_key = inp["w_key"].reshape(C, C // 4, KS)          # (o, i_local, tap)
    wk = np.zeros((KS, C, C), f)
    for o in range(C):
        g = o // 32
        wk[:, 32 * g:32 * (g + 1), o] = (w_key[o].T * s_k[o])
    d["wkey"] = np.ascontiguousarray(wk.transpose(1, 0, 2))  # (i, tap, o)
    d["tk"] = t_k.reshape(C, 1)

    s_e = (inp["bne_g"] / np.sqrt(inp["bne_v"] + EPS)).astype(f)
    t_e = (inp["bne_b"] - inp["bne_m"] * s_e).astype(f)
    we1 = inp["w_e1"] * s_e[:, None]                      # (64, 256)
    d["we1a"] = np.ascontiguousarray(we1[:, :C].T).astype(f)
    d["we1b"] = np.ascontiguousarray(we1[:, C:].T).astype(np.float16)
    d["te"] = t_e.reshape(C // 2, 1)
    d["we2"] = np.ascontiguousarray(inp["w_e2"].T).astype(np.float16)

    s_1 = (inp["bn1_g"] / np.sqrt(inp["bn1_v"] + EPS)).astype(f)
    t_1 = (inp["bn1_b"] - inp["bn1_m"] * s_1).astype(f)
    d["wv"] = np.ascontiguousarray((inp["w_1x1"] * s_1[:, None]).T).astype(f)
    d["tv"] = t_1.reshape(C, 1)

    gn_g, gn_b, b_e2 = inp["gn_g"], inp["gn_b"], inp["b_e2"]
    rows = np.arange(KC)
    cols = np.arange(C)
    bkg = np.zeros((KS, KC, C), f)
    for k in range(KS):
        bkg[k] = (rows[:, None] == (3 * (cols[None, :] // 4) + k)) * \
            gn_g[rows][:, None]
    d["bkg"] = np.ascontiguousarray(
        bkg.transpose(1, 0, 2)).astype(np.float16)   # (r, k, c)
    d["gmu"] = ((rows[:, None] // 3 == np.arange(G)[None, :]) /
                np.float32(KS * H * W)).astype(f)
    d["c1"] = (np.arange(G)[:, None] == (cols[None, :] // 4)).astype(f)
    d["c96"] = (np.arange(G)[:, None] == (rows[None, :] // 3)).astype(f)
    d["badj"] = np.stack([PXC * b_e2, 2.0 * b_e2, PXC * b_e2 * b_e2],
                         axis=1).astype(f)
    ge = 3 * (cols // 4)
    ecols = np.zeros((C, KS, 3), f)
    for k in range(KS):
        ecols[:, k, 0] = b_e2[ge + k]
        ecols[:, k, 1] = gn_g[ge + k]
        ecols[:, k, 2] = gn_b[ge + k]
    d["ecols"] = ecols

    s_2 = (inp["bn2_g"] / np.sqrt(inp["bn2_v"] + EPS)).astype(f)
    d["s2"] = s_2.reshape(C, 1)
    d["t2"] = (inp["bn2_b"] - inp["bn2_m"] * s_2).astype(f).reshape(C, 1)

    s_se = (inp["bnse_g"] / np.sqrt(inp["bnse_v"] + EPS)).astype(f)
    # gap in reference = mean over (N,H,W) of sum over 2 branches; our gap
    # accumulates the raw sum of (agg+k2d) over the clip = 32768 terms
    wse1 = inp["w_se1"] * (s_se[:, None] / np.float32(NF * H * W))
    d["wse1"] = np.ascontiguousarray(wse1.T).astype(f)
    d["b1"] = (s_se * inp["b_se1"] +
               (inp["bnse_b"] - inp["bnse_m"] * s_se)).astype(f).reshape(C, 1)
    w2 = inp["w_se2"]
    d["wsed"] = np.ascontiguousarray((w2[0::2, :] - w2[1::2, :]).T).astype(f)
    d["bd"] = (inp["b_se2"][0::2] - inp["b_se2"][1::2]).astype(f).reshape(C, 1)
    d["ident"] = np.eye(C, dtype=np.float16)
    d["epsv"] = np.full((G, 1), EPS, f)
    return d


def kernel(**inputs):
    if "nc" not in _CACHE:
        _CACHE["nc"] = _build_program()
    nc = _CACHE["nc"]

    consts = _host_constants(inputs)
    x = np.ascontiguousarray(inputs["x"].astype(np.float32))
    x5 = x.reshape(B, NF, C, H, W)

    in_maps = []
    for core in range(NCORES):
        clip, q = core // 2, core % 2
        xs = x5[clip][:, :, q * 32:(q + 1) * 32, :]          # (NF, C, 32, W)
        xs = np.ascontiguousarray(
            xs.transpose(1, 0, 2, 3).reshape(C, NF, PXC))
        m = dict(consts)
        m["x"] = xs
        in_maps.append(m)

    res = run_bass_kernel_spmd(nc, in_maps, list(range(NCORES)))

    out = np.empty((B, NF, C, H, W), np.float32)
    for core in range(NCORES):
        clip, q = core // 2, core % 2
        o = res.results[core]["out"].reshape(C, NF, 32, W)
        out[clip][:, :, q * 32:(q + 1) * 32, :] = o.transpose(1, 0, 2, 3)
    return out.reshape(B * NF, C, H, W)


if __name__ == "__main__":
    sys.path.insert(0, "/root/problem")
    import reference
    inp = {k: np.asarray(v) for k, v in reference.setup_inputs().items()}
    got = kernel(**inp)
    exp = np.asarray(reference.reference(**inp))
    err = np.abs(got - exp).max() / np.abs(exp).max()
    print("abs-max relative error:", err)
